# revision 13
# baseline (speedup 1.0000x reference)
"""CollisionRegularizer loss on 8 Trainium2 cores — v2.

Key ideas vs the v1 baseline (408us):
  * r_directional^2 for both pair orderings is computed DIRECTLY as a
    K=30 bilinear form on the PE (quadratic-form features prepped on
    host), eliminating the 6 projection matmuls + 6 ACT squares + 4 DVE
    adds per tile of v1.
  * Pair matrix is symmetric (spec and the masked-velocity integrand are
    both symmetric in (n,m)), so only the upper block-triangle is
    computed: row-tile k covers columns 128k..2048.  Off-diagonal blocks
    are counted twice on the host; true diagonal is zeroed with a
    128x128 (1-I) mask multiply on the fp16 chain.
  * Slow `nc.vector.reciprocal` (iterative divide) replaced by the
    single-instruction RECIPROCAL_APPROX_FAST custom DVE op (fp16
    in/out via direct _custom_dve emission).
  * spec/vt accumulations ride scalar_tensor_tensor accum_out (4x-mode
    capable InstTensorScalarPtr).
  * All 8 cores run ONE program with fixed slot widths W(ti)=2048-256*ti;
    odd row-halves are shifted by 128 columns by the host and see 128
    sentinel pad columns (distant points, zero scales/velocities) that
    contribute exactly 0.

Sharding: core c -> batch c//2, row-half c%2; slot ti -> row-tile
k = 2*ti + half (rows 128k..128k+128).
"""

import numpy as np

import concourse.bacc as bacc
import concourse.mybir as mybir
from concourse import tile

B, N = 4, 2048
NC = 8
NSLOT = 8
ROWS = 1024           # 8 row-tiles of 128 per core
PADC = 256            # sentinel pad columns appended to rhs (global)
RHS_COLS = 2176       # per-core rhs view width
S16 = 1.0 / 16.0      # r1s/r2s prescale so fp16 stays in range
D2_BIAS = 1e-4        # distance^2 floor (replaces diagonal clamp)

F32 = mybir.dt.float32
F32R = mybir.dt.float32r
F16 = mybir.dt.float16

# engine assignment for tunable ops: 'dve' | 'act' | 'pool'
ASSIGN = {
    "r1c": "act",    # clamp of r1s psum (relu)
    "r2c": "dve",    # clamp of r2s psum (tensor_scalar max)
    "rsum": "pool",
    "ovp": "pool",
    "t": "dve",
    "g": "dve",
    "den": "dve",
}
FOLD = True  # reduce spec/vt tiles via ones^T matmuls into PSUM strips
DRAIN_BUFS = 2   # ring depth for PE->chain handoff tiles
CHAIN_BUFS = 6   # ring depth for chunk-granular chain tiles


def _slot_w(ti):
    return 2048 - 256 * ti


def _chunks(w):
    out = []
    off = 0
    while off < w:
        cw = min(512, w - off)
        out.append((off, cw))
        off += cw
    return out


def _quat_to_rotmat(q):
    qw, qx, qy, qz = q[..., 0], q[..., 1], q[..., 2], q[..., 3]
    R = np.stack(
        [
            1 - 2 * qy**2 - 2 * qz**2, 2 * qx * qy - 2 * qz * qw, 2 * qx * qz + 2 * qy * qw,
            2 * qx * qy + 2 * qz * qw, 1 - 2 * qx**2 - 2 * qz**2, 2 * qy * qz - 2 * qx * qw,
            2 * qx * qz - 2 * qy * qw, 2 * qy * qz + 2 * qx * qw, 1 - 2 * qx**2 - 2 * qy**2,
        ],
        axis=-1,
    )
    return R.reshape(*q.shape[:-1], 3, 3)


def _xx6(x):
    return np.stack(
        [x[:, 0] * x[:, 0], x[:, 0] * x[:, 1], x[:, 0] * x[:, 2],
         x[:, 1] * x[:, 1], x[:, 1] * x[:, 2], x[:, 2] * x[:, 2]], 0)


def _rr10(Rcol, aj):
    # [RR6 with doubled cross terms, -2*aj*Rcol, aj^2] -> (10, N)
    r0, r1, r2 = Rcol[:, 0], Rcol[:, 1], Rcol[:, 2]
    return np.stack(
        [r0 * r0, 2 * r0 * r1, 2 * r0 * r2, r1 * r1, 2 * r1 * r2, r2 * r2,
         -2 * aj * r0, -2 * aj * r1, -2 * aj * r2, aj * aj], 0)


def _sfeat10(x, s2j):
    # [s2j*xx6, s2j*x, s2j] -> (10, N)
    return np.concatenate([s2j * _xx6(x), s2j * x.T, s2j[None, :] * np.ones((1, x.shape[0]))], 0)


def _prep(xyz, scales, rotations, velocities):
    """Per-batch padded feature stacks (float64 host math, f32 out)."""
    NB = N + PADC
    rhs_d2 = np.zeros((B, 5, NB), np.float64)
    rhs_va = np.zeros((B, 8, NB), np.float64)
    rhs_r1 = np.zeros((B, 30, NB), np.float64)
    rhs_r2 = np.zeros((B, 30, NB), np.float64)
    lhs_d2 = np.zeros((B, 5, N), np.float64)
    lhs_va = np.zeros((B, 8, N), np.float64)
    lhs_r1 = np.zeros((B, 30, N), np.float64)
    lhs_r2 = np.zeros((B, 30, N), np.float64)

    # sentinel pad point: far away, zero scales/velocities, identity R
    xp = np.zeros((PADC, 3)); xp[:, 0] = 80.0
    sp = np.zeros((PADC, 3)); vp = np.zeros((PADC, 3))
    Rp = np.broadcast_to(np.eye(3), (PADC, 3, 3))

    for b in range(B):
        x = xyz[b].astype(np.float64)
        s = scales[b].astype(np.float64)
        v = velocities[b].astype(np.float64)
        R = _quat_to_rotmat(rotations[b].astype(np.float64))

        xa = np.concatenate([x, xp], 0)
        sa = np.concatenate([s, sp], 0)
        va_ = np.concatenate([v, vp], 0)
        Ra = np.concatenate([R, Rp], 0)

        aa = np.einsum("ni,nij->nj", xa, Ra)
        ca = (va_ * xa).sum(-1)
        nrma = (xa * xa).sum(-1)
        s2a = sa * sa

        rhs_d2[b, 0:3] = xa.T
        rhs_d2[b, 3] = 1.0
        rhs_d2[b, 4] = nrma
        rhs_va[b, 0:3] = xa.T
        rhs_va[b, 3] = 1.0
        rhs_va[b, 4:7] = va_.T
        rhs_va[b, 7] = ca
        for j in range(3):
            rhs_r1[b, 10 * j:10 * j + 10] = _sfeat10(xa, s2a[:, j])
            rhs_r2[b, 10 * j:10 * j + 10] = _rr10(Ra[:, :, j], aa[:, j])

        a = aa[:N]; c = ca[:N]; nrm = nrma[:N]; s2 = s2a[:N]
        lhs_d2[b, 0:3] = -2.0 * x.T
        lhs_d2[b, 3] = nrm + D2_BIAS
        lhs_d2[b, 4] = 1.0
        lhs_va[b, 0:3] = v.T
        lhs_va[b, 3] = -c
        lhs_va[b, 4:7] = x.T
        lhs_va[b, 7] = -1.0
        for j in range(3):
            lhs_r1[b, 10 * j:10 * j + 10] = _rr10(R[:, :, j], a[:, j]) * S16
            lhs_r2[b, 10 * j:10 * j + 10] = _sfeat10(x, s2[:, j]) * S16

    f = np.float32
    return (f(rhs_d2), f(rhs_va), f(rhs_r1), f(rhs_r2),
            f(lhs_d2), f(lhs_va), f(lhs_r1), f(lhs_r2))


_NC_CACHE = {}


def _build(reps=1):
    key = (reps, FOLD, DRAIN_BUFS, CHAIN_BUFS, tuple(sorted(ASSIGN.items())))
    if key in _NC_CACHE:
        return _NC_CACHE[key]
    AF = mybir.ActivationFunctionType
    ALU = mybir.AluOpType
    from concourse.dve_ops import (
        RECIP_APPROX_FAST_CONSTS as RC,
        RECIPROCAL_APPROX_FAST,
    )
    nc = bacc.Bacc(None, target_bir_lowering=False, debug=False)

    rhs_d2_d = nc.dram_tensor("rhs_d2", [5, RHS_COLS], F32, kind="ExternalInput")
    rhs_va_d = nc.dram_tensor("rhs_va", [8, RHS_COLS], F32R, kind="ExternalInput")
    rhs_r1_d = nc.dram_tensor("rhs_r1", [30, RHS_COLS], F32R, kind="ExternalInput")
    rhs_r2_d = nc.dram_tensor("rhs_r2", [30, RHS_COLS], F32R, kind="ExternalInput")
    lhs_d2_d = nc.dram_tensor("lhs_d2", [5, ROWS], F32, kind="ExternalInput")
    lhs_va_d = nc.dram_tensor("lhs_va", [8, ROWS], F32R, kind="ExternalInput")
    lhs_r1_d = nc.dram_tensor("lhs_r1", [30, ROWS], F32R, kind="ExternalInput")
    lhs_r2_d = nc.dram_tensor("lhs_r2", [30, ROWS], F32R, kind="ExternalInput")
    dmask_d = nc.dram_tensor("dmask", [128, 128], F16, kind="ExternalInput")
    onesc_d = nc.dram_tensor("onesc", [128, 3], F16, kind="ExternalInput")
    if FOLD:
        out_d = nc.dram_tensor("out", [1, 1024], F32, kind="ExternalOutput")
    else:
        out_d = nc.dram_tensor("out", [128, 4 * NSLOT], F32, kind="ExternalOutput")

    def _recip_fast(eng, out, in_):
        return eng._custom_dve(
            RECIPROCAL_APPROX_FAST, out=out, in0=in_,
            s0=RC["s0"], s1=RC["s1"], imm2=RC["imm2"])

    with tile.TileContext(nc) as tc:
        with (
            tc.tile_pool(name="io", bufs=1) as io,
            tc.tile_pool(name="wk", bufs=2) as wk,
            tc.tile_pool(name="ps", bufs=8, space="PSUM") as ps,
        ):
            rhs_d2_s = io.tile([5, RHS_COLS], F32)
            nc.sync.dma_start(rhs_d2_s[:], rhs_d2_d[:])
            rhs_va_s = io.tile([8, RHS_COLS], F32R)
            nc.sync.dma_start(rhs_va_s[:], rhs_va_d[:])
            rhs_r1_s = io.tile([30, RHS_COLS], F32R)
            nc.sync.dma_start(rhs_r1_s[:], rhs_r1_d[:])
            rhs_r2_s = io.tile([30, RHS_COLS], F32R)
            nc.sync.dma_start(rhs_r2_s[:], rhs_r2_d[:])
            lhs_d2_s = io.tile([5, ROWS], F32)
            nc.sync.dma_start(lhs_d2_s[:], lhs_d2_d[:])
            lhs_va_s = io.tile([8, ROWS], F32R)
            nc.sync.dma_start(lhs_va_s[:], lhs_va_d[:])
            lhs_r1_s = io.tile([30, ROWS], F32R)
            nc.sync.dma_start(lhs_r1_s[:], lhs_r1_d[:])
            lhs_r2_s = io.tile([30, ROWS], F32R)
            nc.sync.dma_start(lhs_r2_s[:], lhs_r2_d[:])
            dmask_s = io.tile([128, 128], F16)
            nc.sync.dma_start(dmask_s[:], dmask_d[:])
            onesc_s = io.tile([128, 3], F16)
            nc.sync.dma_start(onesc_s[:], onesc_d[:])
            if FOLD:
                ostrip = io.tile([1, 1024], F32)
                fd = ps.tile([1, 512], F32, name="fd", tag="fd", bufs=1)
                fo = ps.tile([1, 512], F32, name="fo", tag="fo", bufs=1)
                # fold-matmul (group-first, group-last) bookkeeping
                nfold_d = 3 * NSLOT
                nfold_o = 3 * sum(max(0, len(_chunks(_slot_w(t))) - 1)
                                  for t in range(NSLOT))
            else:
                ocols = io.tile([128, 4 * NSLOT], F32)

            from contextlib import nullcontext
            loop_cm = tc.For_i(0, reps, 1) if reps > 1 else nullcontext()
            with loop_cm:
              ifold_d, ifold_o = 0, 0
              for ti in range(NSLOT):
                W = _slot_w(ti)
                rsl = slice(ti * 128, ti * 128 + 128)
                cbase = 256 * ti
                mmbufs = 6 if FOLD else 8

                for ci, (off, cw) in enumerate(_chunks(W)):
                    gsl = slice(cbase + off, cbase + off + cw)
                    pd2 = ps.tile([128, cw], F32, name="pd2", tag="mm", bufs=mmbufs)
                    nc.tensor.matmul(pd2[:], lhs_d2_s[:, rsl], rhs_d2_s[:, gsl],
                                     start=True, stop=True)
                    pr1 = ps.tile([128, cw], F32, name="pr1", tag="mm", bufs=mmbufs)
                    nc.tensor.matmul(pr1[:], lhs_r1_s[:, rsl], rhs_r1_s[:, gsl],
                                     start=True, stop=True)
                    pr2 = ps.tile([128, cw], F32, name="pr2", tag="mm", bufs=mmbufs)
                    nc.tensor.matmul(pr2[:], lhs_r2_s[:, rsl], rhs_r2_s[:, gsl],
                                     start=True, stop=True)
                    pva = ps.tile([128, cw], F32, name="pva", tag="mm", bufs=mmbufs)
                    nc.tensor.matmul(pva[:], lhs_va_s[:, rsl], rhs_va_s[:, gsl],
                                     start=True, stop=True)

                    dist = wk.tile([128, cw], F16, name="dist", tag="dist", bufs=CHAIN_BUFS)
                    nc.scalar.activation(dist[:], pd2[:], AF.Sqrt)
                    r1c = wk.tile([128, cw], F16, name="r1c", tag="r1c", bufs=CHAIN_BUFS)
                    if ASSIGN["r1c"] == "act":
                        nc.scalar.activation(r1c[:], pr1[:], AF.Relu)
                    else:
                        nc.vector.tensor_scalar_max(r1c[:], pr1[:], 0.0)
                    r2c = wk.tile([128, cw], F16, name="r2c", tag="r2c", bufs=CHAIN_BUFS)
                    if ASSIGN["r2c"] == "act":
                        nc.scalar.activation(r2c[:], pr2[:], AF.Relu)
                    else:
                        nc.vector.tensor_scalar_max(r2c[:], pr2[:], 0.0)
                    rva = wk.tile([128, cw], F16, name="rva", tag="rva", bufs=CHAIN_BUFS)
                    nc.scalar.activation(rva[:], pva[:], AF.Relu, scale=0.1)

                    r1 = wk.tile([128, cw], F16, name="r1", tag="r1", bufs=CHAIN_BUFS)
                    nc.scalar.activation(r1[:], r1c[:], AF.Sqrt, scale=16.0)
                    r2 = wk.tile([128, cw], F16, name="r2", tag="r2", bufs=CHAIN_BUFS)
                    nc.scalar.activation(r2[:], r2c[:], AF.Sqrt, scale=16.0)
                    inv = wk.tile([128, cw], F16, name="inv", tag="inv", bufs=CHAIN_BUFS)
                    with nc.allow_low_precision("fp16 chain, ~51ULP recip"):
                        _recip_fast(nc.vector, inv[:], dist[:])

                    _eng = lambda k: {"dve": nc.vector, "pool": nc.gpsimd}[ASSIGN[k]]
                    rsum = wk.tile([128, cw], F16, name="rsum", tag="rsum", bufs=CHAIN_BUFS)
                    _eng("rsum").tensor_add(rsum[:], r1[:], r2[:])
                    t = wk.tile([128, cw], F16, name="t", tag="t", bufs=CHAIN_BUFS)
                    _eng("t").tensor_mul(t[:], rsum[:], inv[:])
                    ovp = wk.tile([128, cw], F16, name="ovp", tag="ovp", bufs=CHAIN_BUFS)
                    _eng("ovp").tensor_sub(ovp[:], t[:], dist[:])
                    ov = wk.tile([128, cw], F16, name="ov", tag="ov", bufs=CHAIN_BUFS)
                    nc.vector.tensor_scalar_max(ov[:], ovp[:], 0.0)
                    if ci == 0:
                        # zero the true diagonal (block-local cols 0..128)
                        nc.vector.tensor_mul(ov[:, 0:128], ov[:, 0:128], dmask_s[:])

                    den = wk.tile([128, cw], F16, name="den", tag="den", bufs=CHAIN_BUFS)
                    if ASSIGN["den"] == "act":
                        nc.scalar.activation(den[:], ov[:], AF.Identity,
                                             bias=1.0, scale=0.1)
                    else:
                        nc.vector.tensor_scalar(den[:], ov[:], 0.1, 1.0,
                                                ALU.mult, ALU.add)
                    rden = wk.tile([128, cw], F16, name="rden", tag="rden", bufs=CHAIN_BUFS)
                    with nc.allow_low_precision("fp16 chain"):
                        _recip_fast(nc.vector, rden[:], den[:])
                    g = wk.tile([128, cw], F16, name="g", tag="g", bufs=CHAIN_BUFS)
                    _eng("g").tensor_mul(g[:], ov[:], inv[:])
                    vt = wk.tile([128, cw], F16, name="vt", tag="vt", bufs=CHAIN_BUFS)
                    nc.vector.tensor_mul(vt[:], g[:], rva[:])

                    # spec = ov^2/(1+0.1ov) == 10*ov - 100 + 100*rden exactly,
                    # so Sum(spec) folds as 10*Sum(ov) + 100*Sum(rden) minus a
                    # static count correction applied on the host.  ones^T
                    # matmuls reduce over partitions into PSUM strips
                    # (fd: chunk-0 incl. diag cols; fo: pure off-diagonal).
                    for src_t, wcol in ((ov, 1), (rden, 2), (vt, 0)):
                        lhs1 = onesc_s[:, wcol:wcol + 1]
                        if ci == 0:
                            nc.tensor.matmul(
                                fd[0:1, 0:cw], lhs1, src_t[:],
                                start=(ifold_d == 0),
                                stop=(ifold_d == nfold_d - 1),
                                skip_group_check=True)
                            ifold_d += 1
                        else:
                            nc.tensor.matmul(
                                fo[0:1, 0:cw], lhs1, src_t[:],
                                start=(ifold_o == 0),
                                stop=(ifold_o == nfold_o - 1),
                                skip_group_check=True)
                            ifold_o += 1

              if FOLD:
                  nc.scalar.activation(ostrip[:, 0:512], fd[0:1, :], AF.Copy)
                  nc.scalar.activation(ostrip[:, 512:1024], fo[0:1, :], AF.Copy)

            if FOLD:
                nc.sync.dma_start(out_d[:], ostrip[:])
            else:
                nc.sync.dma_start(out_d[:], ocols[:])

    nc.compile()
    _NC_CACHE[key] = nc
    return nc


def make_in_maps(xyz, scales, rotations, velocities):
    rhs_d2, rhs_va, rhs_r1, rhs_r2, lhs_d2, lhs_va, lhs_r1, lhs_r2 = _prep(
        xyz, scales, rotations, velocities)
    dmask = (1.0 - np.eye(128)).astype(np.float16)
    in_maps = []
    for c in range(NC):
        b, half = c // 2, c % 2
        csl = slice(128 * half, 128 * half + RHS_COLS)
        # lhs rows: slot ti -> row-tile k = 2*ti + half
        def lrows(a):
            return np.ascontiguousarray(np.concatenate(
                [a[:, 128 * (2 * ti + half):128 * (2 * ti + half) + 128]
                 for ti in range(NSLOT)], axis=1))
        in_maps.append({
            "rhs_d2": np.ascontiguousarray(rhs_d2[b][:, csl]),
            "rhs_va": np.ascontiguousarray(rhs_va[b][:, csl]),
            "rhs_r1": np.ascontiguousarray(rhs_r1[b][:, csl]),
            "rhs_r2": np.ascontiguousarray(rhs_r2[b][:, csl]),
            "lhs_d2": lrows(lhs_d2[b]),
            "lhs_va": lrows(lhs_va[b]),
            "lhs_r1": lrows(lhs_r1[b]),
            "lhs_r2": lrows(lhs_r2[b]),
            "dmask": dmask,
            "onesc": np.broadcast_to(np.array([1.0, 10.0, 100.0], np.float16), (128, 3)).copy(),
        })
    return in_maps


def finish(results):
    total = 0.0
    for c in range(NC):
        o = results[c]["out"].astype(np.float64)
        if FOLD:
            d, off = o[0, 0:512], o[0, 512:1024]
            total += d[0:128].sum() + 2.0 * (d[128:512].sum() + off.sum())
            cw0 = [min(512, _slot_w(t)) for t in range(NSLOT)]
            n_d1 = 128 * 128 * NSLOT
            n_d2 = 128 * sum(c - 128 for c in cw0)
            n_o = 128 * sum(_slot_w(t) - c for t, c in zip(range(NSLOT), cw0))
            total -= 100.0 * (n_d1 + 2.0 * (n_d2 + n_o))
        else:
            for ti in range(NSLOT):
                total += o[:, 4 * ti].sum() + 2.0 * o[:, 4 * ti + 1].sum()
                total += o[:, 4 * ti + 2].sum() + 2.0 * o[:, 4 * ti + 3].sum()
    return np.float32(total / (B * N * N))


_RUNNER = {}


def _get_runner(reps=1):
    """Cached shard_map-jitted executor (mirrors bass2jax.run_bass_via_pjrt
    multi-core path) so repeated calls skip re-compilation."""
    if reps in _RUNNER:
        return _RUNNER[reps]
    import jax
    from jax.sharding import Mesh, PartitionSpec
    from jax.experimental.shard_map import shard_map
    from concourse import bass2jax

    nc = _build(reps)
    bass2jax.install_neuronx_cc_hook()

    part_name = nc.partition_id_tensor.name if nc.partition_id_tensor else None
    in_names, out_names, out_avals, zero_outs = [], [], [], []
    for alloc in nc.m.functions[0].allocations:
        if not isinstance(alloc, mybir.MemoryLocationSet):
            continue
        name = alloc.memorylocations[0].name
        if alloc.kind == "ExternalInput":
            if name != part_name:
                in_names.append(name)
        elif alloc.kind == "ExternalOutput":
            out_names.append(name)
            shape = tuple(alloc.tensor_shape)
            dtype = mybir.dt.np(alloc.dtype)
            out_avals.append(jax.core.ShapedArray(shape, dtype))
            zero_outs.append(np.zeros(shape, dtype))
    n_params = len(in_names)
    all_names = in_names + out_names
    if part_name is not None:
        all_names = all_names + [part_name]

    def _body(*args):
        operands = list(args)
        if part_name is not None:
            operands.append(bass2jax.partition_id_tensor())
        outs = bass2jax._bass_exec_p.bind(
            *operands,
            out_avals=tuple(out_avals),
            in_names=tuple(all_names),
            out_names=tuple(out_names),
            lowering_input_output_aliases=(),
            sim_require_finite=True,
            sim_require_nnan=True,
            nc=nc,
        )
        return tuple(outs)

    devices = jax.devices()[:NC]
    mesh = Mesh(np.asarray(devices), ("core",))
    n_outs = len(out_names)
    fn = jax.jit(
        shard_map(
            _body, mesh=mesh,
            in_specs=(PartitionSpec("core"),) * (n_params + n_outs),
            out_specs=(PartitionSpec("core"),) * n_outs,
            check_rep=False,
        ),
        donate_argnums=tuple(range(n_params, n_params + n_outs)),
        keep_unused=True,
    )

    def run(in_maps):
        concat_in = [
            np.concatenate([in_maps[c][nm] for c in range(NC)], axis=0)
            for nm in in_names
        ]
        concat_zeros = [
            np.zeros((NC * z.shape[0], *z.shape[1:]), z.dtype) for z in zero_outs
        ]
        out_arrs = fn(*concat_in, *concat_zeros)
        return [
            {nm: np.asarray(out_arrs[i]).reshape(NC, *out_avals[i].shape)[c]
             for i, nm in enumerate(out_names)}
            for c in range(NC)
        ]

    _RUNNER[reps] = run
    return run


def kernel(xyz, scales, rotations, velocities):
    run = _get_runner()
    in_maps = make_in_maps(xyz, scales, rotations, velocities)
    return finish(run(in_maps))


if __name__ == "__main__":
    rng = np.random.default_rng(0)
    ins = {
        "xyz": rng.standard_normal((B, N, 3)).astype(np.float32),
        "scales": rng.random((B, N, 3)).astype(np.float32),
        "rotations": rng.standard_normal((B, N, 4)).astype(np.float32),
        "velocities": rng.standard_normal((B, N, 3)).astype(np.float32),
    }
    print(kernel(**ins))
